# revision 2
# baseline (speedup 1.0000x reference)
"""GraphSAGE 2-layer GNN on 8 Trainium2 NeuronCores (Bass/Tile).

Strategy (matches the sharding hint):
  - Nodes sharded across 8 cores (12544 = 98*128 padded rows each).
  - Edges partitioned by destination core, so the segment-sum is local.
  - Full x replicated per core (layer-1 gather table).  The layer-2 gather
    table y2 = h1 @ Wl2 is computed per-slice and AllGather'ed on device.
  - Aggregation: dma_gather pulls source rows into SBUF [128 edges, 128 feat]
    chunks; a per-chunk selection matrix S (S[e, n] = inv_deg * (slot[e]==n),
    built on DVE from iota/slot/w data) turns segment-sum into PE matmuls
    accumulating in PSUM, in transposed [feat, node] layout.
  - out^T = aggT + W^T @ (in^T) (+ bias per partition), ReLU on ACT.

Self-contained: hardcodes shapes for the nn_GraphSAGENet problem
(x [100000,128] f32, edge_index [2,1600000] int64, 128x128 weights).
"""

import sys

sys.path.insert(0, "/opt/trn_rl_repo")

import numpy as np

N_NODES = 100000
F = 128
N_CORES = 8
NPC = 12544             # nodes per core (98 * 128)
TILES = NPC // 128      # 98
NPAD = N_CORES * NPC    # 100352
N_CHUNKS = 4
CHUNK_ROWS = 25088      # source-index chunk (int16 range)
GROUP_CAP = 640         # max edges per (core, tile, chunk-group), 5x128
SUB = GROUP_CAP // 128  # 5 sub-chunks of 128 edges per group
CH = N_CHUNKS * SUB     # 20 chunks of 128 edge-slots per tile
IDX_COLS = GROUP_CAP // 16  # 40

_compiled = None


def _build_program():
    import concourse.bacc as bacc
    import concourse.mybir as mybir
    import concourse.tile as tile
    from concourse import library_config

    f32 = mybir.dt.float32
    i16 = mybir.dt.int16
    Alu = mybir.AluOpType
    Act = mybir.ActivationFunctionType

    nc = bacc.Bacc("TRN2", target_bir_lowering=False)

    x_d = nc.dram_tensor("x", [N_NODES, F], f32, kind="ExternalInput")
    xT_d = nc.dram_tensor("xT", [F, NPC], f32, kind="ExternalInput")
    mi_d = nc.dram_tensor("meta_i16", [TILES, 128, N_CHUNKS * IDX_COLS], i16,
                          kind="ExternalInput")
    mf_d = nc.dram_tensor("meta_f32", [TILES, 128, 2 * CH], f32,
                          kind="ExternalInput")
    wl1_d = nc.dram_tensor("Wl1", [F, F], f32, kind="ExternalInput")
    wr1_d = nc.dram_tensor("Wr1", [F, F], f32, kind="ExternalInput")
    wl2_d = nc.dram_tensor("Wl2", [F, F], f32, kind="ExternalInput")
    wr2_d = nc.dram_tensor("Wr2", [F, F], f32, kind="ExternalInput")
    b1_d = nc.dram_tensor("b1", [F, 1], f32, kind="ExternalInput")
    b2_d = nc.dram_tensor("b2", [F, 1], f32, kind="ExternalInput")
    iota_d = nc.dram_tensor("iota", [128, 128], f32, kind="ExternalInput")
    ident_d = nc.dram_tensor("ident", [128, 128], f32, kind="ExternalInput")
    out_d = nc.dram_tensor("out", [NPC, F], f32, kind="ExternalOutput")

    y2s_d = nc.dram_tensor("y2_slice", [NPC, F], f32)
    y2f_d = nc.dram_tensor("y2_full", [NPAD, F], f32)

    def chunk_rows(g, total_rows):
        lo = g * CHUNK_ROWS
        hi = min((g + 1) * CHUNK_ROWS, total_rows)
        return lo, hi

    with tile.TileContext(nc) as tc:
        with tc.tile_pool(name="const", bufs=1) as cpool:
            nc.gpsimd.load_library(library_config.mlp)
            wl1_t = cpool.tile([F, F], f32)
            wr1_t = cpool.tile([F, F], f32)
            wl2_t = cpool.tile([F, F], f32)
            wr2_t = cpool.tile([F, F], f32)
            b1_t = cpool.tile([F, 1], f32)
            b2_t = cpool.tile([F, 1], f32)
            iota_t = cpool.tile([128, 128], f32)
            ident_t = cpool.tile([128, 128], f32)
            h1T = cpool.tile([F, NPC], f32)  # resident hidden activations^T
            for t_, d_ in [(wl1_t, wl1_d), (wr1_t, wr1_d), (wl2_t, wl2_d),
                           (wr2_t, wr2_d), (b1_t, b1_d), (b2_t, b2_d),
                           (iota_t, iota_d), (ident_t, ident_d)]:
                nc.sync.dma_start(t_[:], d_[:])

            def aggregate(pool, spool, ppool, t, table_d, table_rows, close):
                """dma_gather + segment-sum matmuls for node-tile t.

                Returns PSUM tile aggT [feat, 128 nodes].  If close is False
                the accumulation group is left open so the caller can
                accumulate one more matmul (which must pass stop=True)."""
                idx_t = pool.tile([128, N_CHUNKS * IDX_COLS], i16, tag="idx")
                mf_t = pool.tile([128, 2 * CH], f32, tag="mf")
                nc.sync.dma_start(idx_t[:], mi_d[t])
                nc.sync.dma_start(mf_t[:], mf_d[t])

                gath = pool.tile([128, CH, F], f32, tag="gath")
                for g in range(N_CHUNKS):
                    lo, hi = chunk_rows(g, table_rows)
                    nc.gpsimd.dma_gather(
                        gath[:, g * SUB:(g + 1) * SUB, :],
                        table_d[lo:hi, :],
                        idx_t[:, g * IDX_COLS:(g + 1) * IDX_COLS],
                        GROUP_CAP, GROUP_CAP, F,
                    )

                psum = ppool.tile([F, 128], f32, tag="agg")
                for j in range(CH):
                    s_t = spool.tile([128, 128], f32, tag="s")
                    nc.vector.tensor_scalar(
                        s_t[:], iota_t[:],
                        mf_t[:, j:j + 1], mf_t[:, CH + j:CH + j + 1],
                        Alu.is_equal, Alu.mult,
                    )
                    nc.tensor.matmul(psum[:], gath[:, j, :], s_t[:],
                                     start=(j == 0),
                                     stop=(close and j == CH - 1))
                return psum

            # ---------------- layer 1 ----------------
            with (
                tc.tile_pool(name="l1", bufs=3) as pool,
                tc.tile_pool(name="l1s", bufs=24) as spool,
                tc.tile_pool(name="l1pa", bufs=2, space="PSUM") as ppool_a,
                tc.tile_pool(name="l1po", bufs=2, space="PSUM") as ppool_o,
                tc.tile_pool(name="l1py", bufs=2, space="PSUM") as ppool_y,
                tc.tile_pool(name="l1pt", bufs=2, space="PSUM") as ppool_t,
            ):
                for t in range(TILES):
                    cols = slice(t * 128, (t + 1) * 128)
                    psum_agg = aggregate(pool, spool, ppool_a, t,
                                         x_d, N_NODES, close=True)
                    # meanT (inv_deg folded into S weights) -> SBUF
                    meanT = pool.tile([F, 128], f32, tag="meanT")
                    nc.scalar.activation(meanT[:], psum_agg[:], Act.Copy)

                    xt_t = pool.tile([F, 128], f32, tag="xt")
                    nc.sync.dma_start(xt_t[:], xT_d[:, cols])

                    # out1T = Wl1^T @ meanT + Wr1^T @ xT
                    psum_out = ppool_o.tile([F, 128], f32, tag="out")
                    nc.tensor.matmul(psum_out[:], wl1_t[:], meanT[:],
                                     start=True, stop=False)
                    nc.tensor.matmul(psum_out[:], wr1_t[:], xt_t[:],
                                     start=False, stop=True)
                    # h1T = relu(out1T + b1), written into resident buffer
                    nc.scalar.activation(h1T[:, cols], psum_out[:],
                                         Act.Relu, bias=b1_t[:])

                    # y2T = Wl2^T @ h1T ; transpose to row-major ; to DRAM
                    psum_y2 = ppool_y.tile([F, 128], f32, tag="y2")
                    nc.tensor.matmul(psum_y2[:], wl2_t[:], h1T[:, cols],
                                     start=True, stop=True)
                    y2T_s = pool.tile([F, 128], f32, tag="y2T")
                    nc.scalar.activation(y2T_s[:], psum_y2[:], Act.Copy)
                    psum_tr = ppool_t.tile([128, F], f32, tag="tr")
                    nc.tensor.transpose(psum_tr[:], y2T_s[:], ident_t[:])
                    y2row = pool.tile([128, F], f32, tag="y2row")
                    nc.vector.tensor_copy(y2row[:], psum_tr[:])
                    nc.sync.dma_start(y2s_d[cols, :], y2row[:])

            # ---------------- all-gather y2 ----------------
            import concourse.mybir as _mb
            nc.gpsimd.collective_compute(
                "AllGather",
                _mb.AluOpType.bypass,
                replica_groups=[list(range(N_CORES))],
                ins=[y2s_d.ap().opt()],
                outs=[y2f_d.ap().opt()],
            )

            # ---------------- layer 2 ----------------
            with (
                tc.tile_pool(name="l2", bufs=3) as pool,
                tc.tile_pool(name="l2s", bufs=24) as spool,
                tc.tile_pool(name="l2pa", bufs=3, space="PSUM") as ppool_a,
                tc.tile_pool(name="l2pt", bufs=3, space="PSUM") as ppool_t,
            ):
                for t in range(TILES):
                    cols = slice(t * 128, (t + 1) * 128)
                    psum_agg = aggregate(pool, spool, ppool_a, t,
                                         y2f_d, NPAD, close=False)
                    # += Wr2^T @ h1T (root term), closes the group
                    nc.tensor.matmul(psum_agg[:], wr2_t[:], h1T[:, cols],
                                     start=False, stop=True)
                    h2T = pool.tile([F, 128], f32, tag="h2T")
                    nc.scalar.activation(h2T[:], psum_agg[:],
                                         Act.Relu, bias=b2_t[:])
                    psum_tr = ppool_t.tile([128, F], f32, tag="tr")
                    nc.tensor.transpose(psum_tr[:], h2T[:], ident_t[:])
                    h2row = pool.tile([128, F], f32, tag="h2row")
                    nc.vector.tensor_copy(h2row[:], psum_tr[:])
                    nc.sync.dma_start(out_d[cols, :], h2row[:])

    nc.compile()
    return nc


def _prep_inputs(x, edge_index, Wl1, Wr1, b1, Wl2, Wr2, b2):
    """Host-side sharding / index packing.  Returns in_maps for 8 cores."""
    x = np.ascontiguousarray(np.asarray(x, np.float32))
    src = np.asarray(edge_index[0], np.int64)
    dst = np.asarray(edge_index[1], np.int64)
    E = src.shape[0]

    deg = np.bincount(dst, minlength=N_NODES).astype(np.float32)
    inv_deg = (1.0 / np.maximum(deg, 1.0)).astype(np.float32)

    core = dst // NPC
    tl = (dst % NPC) // 128
    grp = src // CHUNK_ROWS
    kk = ((core * TILES) + tl) * N_CHUNKS + grp
    NG = N_CORES * TILES * N_CHUNKS

    order = np.argsort(kk, kind="stable")
    counts = np.bincount(kk, minlength=NG)
    mx = counts.max()
    assert mx <= GROUP_CAP, f"group overflow: {mx} > {GROUP_CAP}"
    starts = np.concatenate(([0], np.cumsum(counts)[:-1]))
    kks = kk[order]
    pos = np.arange(E) - starts[kks]
    tgt = kks * GROUP_CAP + pos

    idx_all = np.zeros(NG * GROUP_CAP, np.int16)
    slot_all = np.full(NG * GROUP_CAP, -1.0, np.float32)
    w_all = np.zeros(NG * GROUP_CAP, np.float32)
    so = src[order]
    do = dst[order]
    idx_all[tgt] = (so - grp[order] * CHUNK_ROWS).astype(np.int16)
    slot_all[tgt] = (do % 128).astype(np.float32)
    w_all[tgt] = inv_deg[do]

    # idx: [C,T,G,640] -> [C,T,G,40,16] -> [C,T,G,16,40] -> bcast 8x -> [C,T,128,G*40]
    idx_r = idx_all.reshape(N_CORES, TILES, N_CHUNKS, IDX_COLS, 16)
    idx_r = idx_r.transpose(0, 1, 2, 4, 3)          # [C,T,G,16,40]
    idx_r = np.broadcast_to(idx_r[:, :, :, None, :, :],
                            (N_CORES, TILES, N_CHUNKS, 8, 16, IDX_COLS))
    idx_r = idx_r.reshape(N_CORES, TILES, N_CHUNKS, 128, IDX_COLS)
    meta_i16 = np.ascontiguousarray(
        idx_r.transpose(0, 1, 3, 2, 4).reshape(N_CORES, TILES, 128,
                                               N_CHUNKS * IDX_COLS))

    # slot/w: [C,T,2560] -> [C,T,20,128] -> [C,T,128,20]
    def to_chunk_layout(a):
        a = a.reshape(N_CORES, TILES, CH, 128)
        return a.transpose(0, 1, 3, 2)
    slotp = to_chunk_layout(slot_all)
    wv = to_chunk_layout(w_all)
    meta_f32 = np.ascontiguousarray(
        np.concatenate([slotp, wv], axis=-1))      # [C,T,128,40]

    xpad = np.zeros((NPAD, F), np.float32)
    xpad[:N_NODES] = x

    iota = np.tile(np.arange(128, dtype=np.float32), (128, 1))
    ident = np.eye(128, dtype=np.float32)
    com = {
        "x": x,
        "Wl1": np.ascontiguousarray(Wl1, dtype=np.float32),
        "Wr1": np.ascontiguousarray(Wr1, dtype=np.float32),
        "Wl2": np.ascontiguousarray(Wl2, dtype=np.float32),
        "Wr2": np.ascontiguousarray(Wr2, dtype=np.float32),
        "b1": np.ascontiguousarray(np.asarray(b1, np.float32).reshape(F, 1)),
        "b2": np.ascontiguousarray(np.asarray(b2, np.float32).reshape(F, 1)),
        "iota": iota,
        "ident": ident,
    }
    in_maps = []
    for c in range(N_CORES):
        m = dict(com)
        m["xT"] = np.ascontiguousarray(xpad[c * NPC:(c + 1) * NPC].T)
        m["meta_i16"] = np.ascontiguousarray(meta_i16[c])
        m["meta_f32"] = np.ascontiguousarray(meta_f32[c])
        in_maps.append(m)
    return in_maps


def _get_compiled():
    global _compiled
    if _compiled is None:
        _compiled = _build_program()
    return _compiled


def run(trace=False, **inputs):
    from concourse.bass_utils import run_bass_kernel_spmd

    nc = _get_compiled()
    in_maps = _prep_inputs(**inputs)
    res = run_bass_kernel_spmd(nc, in_maps, list(range(N_CORES)), trace=trace)
    out = np.concatenate([res.results[c]["out"] for c in range(N_CORES)], axis=0)
    return out[:N_NODES], res


def kernel(**inputs):
    out, _ = run(trace=False, **inputs)
    return out
